# revision 26
# baseline (speedup 1.0000x reference)
"""2D DCT-II (4096x4096) on 8 Trainium2 NeuronCores (axon/PJRT SPMD).

Math: the reference computes C = A_M @ x @ A_N^T where the Makhoul even-odd
permutation is folded into dense tables built from the expk inputs.  Folding
reconstitutes the *standard* DCT-II matrix in natural input order:
  A_M[u, m] = 0.5*cos(pi*u*(2m+1)/(2N)),  A_N^T[c, v] = 2*cos(pi*v*(2c+1)/(2N))
which obeys the classic even-odd split: even (odd) output rows are symmetric
(antisymmetric) under m -> N-1-m.  So with mirror butterflies
  e[m] = x[m] + x[N-1-m],  o[m] = x[m] - x[N-1-m]   (m < N/2)
each dense 4096-point transform becomes two independent 2048-point GEMMs
against the even/odd column halves of the original tables - half the PE work
and half the table traffic of the direct form.

Distribution (8 cores), with no transposes anywhere (host or device):
  core k holds x[:, cols_k].
  phase 1: row butterflies (mirror partner obtained losslessly via a PE
           matmul with the antidiagonal identity J), then
           ZT_even = e^T @ g2T, ZT_odd = o^T @ g4T  ->  Z^T[cols_k, :] in
           even/odd-of-u order; written as 8 [512, 512] blocks where block j
           holds the k'-ranges whose true u rows land on core j
           (slots [0:256) = even u, [256:512) = odd u).
  AllToAll: block j of core k -> core j; core k then holds Z^T[:, rows_k]
           (u in slot order) with the c dimension natural.
  phase 2: column butterflies on c (same J trick), then
           C_even_v = eZ^T @ h2T, C_odd_v = oZ^T @ h4T; the final evacuation
           interleaves even/odd v via stride-2 DVE writes and lands on
           stride-2 row slices of cout, undoing the slot order for free.
Host: x uploads column-sharded (bf16), C returns row-sharded (bf16) as one
global jax array - zero host-side reshuffling.  Tables upload once
(replicated, 32 MB total) and stay cached on device.

Everything is bf16 except PSUM accumulation (fp32).  Measured end-to-end
rel err ~5e-3 vs the f64 reference (gate is 2e-2).
"""
import numpy as np

_NCORES = 8
_SZ = 4096
_H = _SZ // 2           # 2048: contraction length after the butterfly
_RPC = _SZ // _NCORES   # 512 rows/cols per core
_KT = _H // 128         # 16 contraction tiles

_state = {}


# --------------------------------------------------------------------------
# Bass kernel
# --------------------------------------------------------------------------
def _build_bass(a2a=True, reps=1):
    import concourse.bacc as bacc
    import concourse.mybir as mybir
    from concourse.tile import TileContext

    fp32 = mybir.dt.float32
    bf16 = mybir.dt.bfloat16
    add = mybir.AluOpType.add
    sub = mybir.AluOpType.subtract
    mult = mybir.AluOpType.mult
    nc = bacc.Bacc("TRN2", target_bir_lowering=False, debug=False,
                   num_devices=_NCORES)
    xc = nc.declare_dram_parameter("xc", [_SZ, _RPC], bf16, isOutput=False)
    # phase-1 tables, pre-tiled: g*[panel, p, kt*1024 + u] = gT[kt*128+p,
    # panel*1024 + u] with gT = amT[:2048, parity::2]
    g2 = nc.declare_dram_parameter("g2", [2, 128, _KT * 1024], bf16,
                                   isOutput=False)
    g4 = nc.declare_dram_parameter("g4", [2, 128, _KT * 1024], bf16,
                                   isOutput=False)
    # phase-2 tables, pre-tiled in 512-wide panels of annT[:2048, parity::2]
    h2 = nc.declare_dram_parameter("h2", [4, 128, _KT * 512], bf16,
                                   isOutput=False)
    h4 = nc.declare_dram_parameter("h4", [4, 128, _KT * 512], bf16,
                                   isOutput=False)
    jrev = nc.declare_dram_parameter("jrev", [128, 128], bf16, isOutput=False)
    cout = nc.declare_dram_parameter("cout", [_RPC, _SZ], bf16, isOutput=True)

    w_send = nc.dram_tensor("w_send", [_NCORES, _RPC, _RPC], bf16)
    w_recv = nc.dram_tensor("w_recv", [_NCORES, _RPC, _RPC], bf16)

    with TileContext(nc) as tc:
      for _rep in range(reps):  # reps>1: timing builds only (slope method)
        # ------------- phase 1: butterfly + ZT = [e;o]^T @ [g2;g4] --------
        with (
            tc.tile_pool(name="xcp", bufs=1) as xc_pool,
            tc.tile_pool(name="eo", bufs=1) as eo_pool,
            tc.tile_pool(name="jp", bufs=1) as j_pool,
            tc.tile_pool(name="gp", bufs=2) as g_pool,
            tc.tile_pool(name="psj", bufs=2, space="PSUM") as psj_pool,
            tc.tile_pool(name="ps1", bufs=6, space="PSUM") as ps1_pool,
            tc.tile_pool(name="ev1", bufs=8) as ev1_pool,
        ):
            jt = j_pool.tile([128, 128], bf16)
            nc.sync.dma_start(out=jt[:], in_=jrev[:])
            xcs = xc_pool.tile([128, 2 * _KT * _RPC], bf16)  # 4 MB
            for q in (0, 3, 1, 2):  # mirror-pair order: butterfly starts
                nc.sync.dma_start(    # after the first two quarter-loads
                    out=xcs[:].rearrange("p (kt v) -> p kt v", kt=2 * _KT)
                    [:, q * 8:(q + 1) * 8, :],
                    in_=xc[q * 1024:(q + 1) * 1024, :]
                    .rearrange("(kt p) v -> p kt v", p=128))
            eT = eo_pool.tile([128, _KT * _RPC], bf16)  # 2 MB
            oT = eo_pool.tile([128, _KT * _RPC], bf16)  # 2 MB
            for kt in range(_KT):
                mir = 2 * _KT - 1 - kt
                pj = psj_pool.tile([128, _RPC], fp32, tag="pj")
                nc.tensor.matmul(
                    pj[:], jt[:],
                    xcs[:, mir * _RPC:(mir + 1) * _RPC],
                    start=True, stop=True)
                nc.vector.scalar_tensor_tensor(
                    out=eT[:, kt * _RPC:(kt + 1) * _RPC],
                    in0=xcs[:, kt * _RPC:(kt + 1) * _RPC],
                    scalar=1.0, in1=pj[:], op0=mult, op1=add)
                nc.vector.scalar_tensor_tensor(
                    out=oT[:, kt * _RPC:(kt + 1) * _RPC],
                    in0=xcs[:, kt * _RPC:(kt + 1) * _RPC],
                    scalar=1.0, in1=pj[:], op0=mult, op1=sub)
            for tab in range(2):  # 0: even u rows (g2,e)  1: odd (g4,o)
                src = eT if tab == 0 else oT
                gparam = g2 if tab == 0 else g4
                slot0 = 0 if tab == 0 else 256
                for panel in range(2):  # k' panels of 1024
                    g = g_pool.tile([128, _KT * 1024], bf16, tag="g")  # 4 MB
                    for q in range(2):
                        nc.sync.dma_start(
                            out=g[:].rearrange("p (kt u) -> p kt u", kt=_KT)
                            [:, q * 8:(q + 1) * 8, :],
                            in_=gparam[panel, :, q * 8 * 1024:
                                       (q + 1) * 8 * 1024]
                            .rearrange("p (kt u) -> p kt u", kt=8))
                    for vt in range(4):
                        for uh in range(2):
                            ps = ps1_pool.tile([128, 512], fp32, tag="ps")
                            for kt in range(_KT):
                                nc.tensor.matmul(
                                    ps[:],
                                    src[:, kt * _RPC + vt * 128:
                                           kt * _RPC + vt * 128 + 128],
                                    g[:, kt * 1024 + uh * 512:
                                         kt * 1024 + (uh + 1) * 512],
                                    start=(kt == 0), stop=(kt == _KT - 1))
                            ev = ev1_pool.tile([128, 512], bf16, tag="ev")
                            nc.scalar.copy(ev[:], ps[:])
                            q_abs = panel * 2 + uh
                            nc.sync.dma_start(
                                out=w_send[2 * q_abs, vt * 128:(vt + 1) * 128,
                                           slot0:slot0 + 256],
                                in_=ev[:, :256])
                            nc.sync.dma_start(
                                out=w_send[2 * q_abs + 1,
                                           vt * 128:(vt + 1) * 128,
                                           slot0:slot0 + 256],
                                in_=ev[:, 256:])

        # ---------- exchange ----------
        if a2a:
            nc.gpsimd.collective_compute(
                "AllToAll",
                mybir.AluOpType.bypass,
                ins=[w_send[:]],
                outs=[w_recv[:]],
                replica_groups=[list(range(_NCORES))],
            )
        else:  # timing-sim variant: same bytes moved, no collective
            nc.sync.dma_start(out=w_recv[:], in_=w_send[:])

        # ------------- phase 2: butterfly on c + C = [eZ;oZ]^T @ [h2;h4] --
        with (
            tc.tile_pool(name="wrp", bufs=1) as wr_pool,
            tc.tile_pool(name="eo2", bufs=1) as eo2_pool,
            tc.tile_pool(name="jp2", bufs=1) as j2_pool,
            tc.tile_pool(name="hp", bufs=4) as h_pool,
            tc.tile_pool(name="psj2", bufs=2, space="PSUM") as psj2_pool,
            tc.tile_pool(name="psE", bufs=3, space="PSUM") as psE_pool,
            tc.tile_pool(name="psO", bufs=3, space="PSUM") as psO_pool,
            tc.tile_pool(name="ev2", bufs=4) as ev2_pool,
        ):
            jt2 = j2_pool.tile([128, 128], bf16)
            nc.sync.dma_start(out=jt2[:], in_=jrev[:])
            wr = wr_pool.tile([128, 2 * _KT * _RPC], bf16)  # 4 MB
            for j in (0, 7, 1, 6, 2, 5, 3, 4):  # mirror-pair order: the
                nc.sync.dma_start(              # butterfly starts after two
                    out=wr[:].rearrange("p (j s u) -> p j s u",
                                        j=_NCORES, s=4)[:, j, :, :],
                    in_=w_recv[j].rearrange("(s p) u -> p s u", p=128))
            eZ = eo2_pool.tile([128, _KT * _RPC], bf16)  # 2 MB
            oZ = eo2_pool.tile([128, _KT * _RPC], bf16)  # 2 MB
            for kt in range(_KT):
                mir = 2 * _KT - 1 - kt
                pj = psj2_pool.tile([128, _RPC], fp32, tag="pj2")
                nc.tensor.matmul(
                    pj[:], jt2[:],
                    wr[:, mir * _RPC:(mir + 1) * _RPC],
                    start=True, stop=True)
                nc.vector.scalar_tensor_tensor(
                    out=eZ[:, kt * _RPC:(kt + 1) * _RPC],
                    in0=wr[:, kt * _RPC:(kt + 1) * _RPC],
                    scalar=1.0, in1=pj[:], op0=mult, op1=add)
                nc.vector.scalar_tensor_tensor(
                    out=oZ[:, kt * _RPC:(kt + 1) * _RPC],
                    in0=wr[:, kt * _RPC:(kt + 1) * _RPC],
                    scalar=1.0, in1=pj[:], op0=mult, op1=sub)
            for panel in range(4):  # k panels of 512
                hc2 = h_pool.tile([128, _KT * 512], bf16, tag="h2")  # 2 MB
                hc4 = h_pool.tile([128, _KT * 512], bf16, tag="h4")  # 2 MB
                nc.sync.dma_start(out=hc2[:], in_=h2[panel])
                nc.sync.dma_start(out=hc4[:], in_=h4[panel])
                for ut in range(4):
                    psE = psE_pool.tile([128, 512], fp32, tag="psE")
                    for kt in range(_KT):
                        nc.tensor.matmul(
                            psE[:],
                            eZ[:, kt * _RPC + ut * 128:
                                  kt * _RPC + ut * 128 + 128],
                            hc2[:, kt * 512:(kt + 1) * 512],
                            start=(kt == 0), stop=(kt == _KT - 1))
                    psO = psO_pool.tile([128, 512], fp32, tag="psO")
                    for kt in range(_KT):
                        nc.tensor.matmul(
                            psO[:],
                            oZ[:, kt * _RPC + ut * 128:
                                  kt * _RPC + ut * 128 + 128],
                            hc4[:, kt * 512:(kt + 1) * 512],
                            start=(kt == 0), stop=(kt == _KT - 1))
                    ev = ev2_pool.tile([128, 1024], bf16, tag="ev2")
                    evs = ev[:].rearrange("p (k two) -> p two k", two=2)
                    nc.vector.tensor_copy(evs[:, 0, :], psE[:])
                    nc.scalar.copy(evs[:, 1, :], psO[:])
                    # u-slot tile -> stride-2 row slice of cout
                    parity, urow = (0, ut) if ut < 2 else (1, ut - 2)
                    nc.sync.dma_start(
                        out=cout[:].rearrange("(u two) v -> two u v", two=2)
                        [parity, urow * 128:(urow + 1) * 128,
                         panel * 1024:(panel + 1) * 1024],
                        in_=ev[:])

    nc.compile()
    return nc


# --------------------------------------------------------------------------
# PJRT SPMD runner (compile once, run many)
# --------------------------------------------------------------------------
def _build_runner(nc, n_cores):
    import jax
    import jax.numpy as jnp
    from jax.sharding import Mesh, PartitionSpec as P, NamedSharding
    from jax.experimental.shard_map import shard_map
    import concourse.mybir as mybir
    from concourse import bass2jax
    from concourse.bass2jax import _bass_exec_p, partition_id_tensor

    bass2jax.install_neuronx_cc_hook()
    partition_name = (nc.partition_id_tensor.name
                      if nc.partition_id_tensor else None)

    # shardings per bass parameter (default: stacked along axis 0 per core)
    param_spec = {
        "xc": P(None, "core"),                   # column shard
        "g2": P(), "g4": P(), "h2": P(), "h4": P(), "jrev": P(),
        "amT": P(), "annT": P(),
    }

    in_names, out_names, out_avals = [], [], []
    for alloc in nc.m.functions[0].allocations:
        if not isinstance(alloc, mybir.MemoryLocationSet):
            continue
        name = alloc.memorylocations[0].name
        if alloc.kind == "ExternalInput":
            if name != partition_name:
                in_names.append(name)
        elif alloc.kind == "ExternalOutput":
            shape = tuple(alloc.tensor_shape)
            dtype = mybir.dt.np(alloc.dtype)
            out_names.append(name)
            out_avals.append(jax.core.ShapedArray(shape, dtype))
    n_outs = len(out_avals)
    in_names_all = list(in_names) + out_names
    if partition_name is not None:
        in_names_all = in_names_all + [partition_name]

    def _body(*args):
        operands = list(args)
        if partition_name is not None:
            operands.append(partition_id_tensor())
        outs = _bass_exec_p.bind(
            *operands,
            out_avals=tuple(out_avals),
            in_names=tuple(in_names_all),
            out_names=tuple(out_names),
            lowering_input_output_aliases=(),
            sim_require_finite=True,
            sim_require_nnan=True,
            nc=nc,
        )
        return tuple(outs)

    devices = jax.devices()[:n_cores]
    mesh = Mesh(np.asarray(devices), ("core",))
    in_specs = tuple(param_spec.get(nm, P("core")) for nm in in_names)
    out_sharding_specs = (P("core"),) * n_outs
    sharded = jax.jit(
        shard_map(_body, mesh=mesh,
                  in_specs=in_specs + out_sharding_specs,
                  out_specs=out_sharding_specs,
                  check_rep=False),
        keep_unused=True)

    out_shard = NamedSharding(mesh, P("core"))
    _dev_cache = {}

    # The "output" operands of the bass_exec custom call are placeholders:
    # the NEFF's result buffers are the custom call's results, so these
    # operands are never consumed. Build them once and reuse every call -
    # one PJRT dispatch per kernel invocation.
    _zero_shapes = [(n_cores * a.shape[0], *a.shape[1:]) for a in out_avals]
    _zero_dtypes = [a.dtype for a in out_avals]
    _make_zeros = jax.jit(
        lambda: tuple(jnp.zeros(s, d)
                      for s, d in zip(_zero_shapes, _zero_dtypes)),
        out_shardings=(out_shard,) * len(_zero_shapes))
    _zeros_cache = []

    def _zeros():
        if not _zeros_cache:
            import jax as _jax
            z = _make_zeros()
            _jax.block_until_ready(z)
            _zeros_cache.append(z)
        return _zeros_cache[0]

    def _put(name, arr):
        import jax as _jax
        spec = param_spec.get(name, P("core"))
        return _jax.device_put(arr, NamedSharding(mesh, spec))

    def run(in_map, cache_names=(), block=True):
        """in_map: full global arrays keyed by bass param name."""
        import jax as _jax
        concat_in = []
        for name in in_names:
            if name in cache_names and name in _dev_cache:
                concat_in.append(_dev_cache[name])
                continue
            darr = _put(name, in_map[name])
            if name in cache_names:
                _jax.block_until_ready(darr)
                _dev_cache[name] = darr
            concat_in.append(darr)
        raw = sharded(*concat_in, *_zeros())
        if block:
            _jax.block_until_ready(raw)
        return raw[0] if n_outs == 1 else raw

    def bench(L):
        """Dispatch L back-to-back executions on cached inputs, block once.
        Returns elapsed wall seconds."""
        import time as _time
        import jax as _jax
        concat_in = [_dev_cache[name] for name in in_names]
        z = _zeros()
        t0 = _time.perf_counter()
        outs = []
        for _ in range(L):
            outs.append(sharded(*concat_in, *z))
        _jax.block_until_ready(outs)
        return _time.perf_counter() - t0

    run.dev_cache = _dev_cache
    run.bench = bench
    run.mesh = mesh
    return run


# --------------------------------------------------------------------------
# host-side tables
# --------------------------------------------------------------------------
def _tables(expkM, expkN):
    import ml_dtypes
    key = (expkM.tobytes(), expkN.tobytes())
    cached = _state.get("tables")
    if cached is not None and cached[0] == key:
        return cached[1]
    run = _state.get("run")
    if run is not None:
        run.dev_cache.clear()
    bf16 = ml_dtypes.bfloat16
    n = _SZ
    i = np.arange(n)
    pm = np.where(i < (n + 1) // 2, 2 * i, 2 * (n - i) - 1)
    pinv = np.empty(n, dtype=np.int64)
    pinv[pm] = i
    # Cp[j, v] = cos(2pi * pinv[j] * v / n); with the permutation folded these
    # are the standard DCT-II tables in natural input order (see module doc).
    ang = (2.0 * np.pi / n) * np.outer(pinv.astype(np.float64),
                                       i.astype(np.float64))
    Cp = np.cos(ang)
    Sp = np.sin(ang)
    eMr = expkM[:, 0].astype(np.float64)
    eMi = expkM[:, 1].astype(np.float64)
    eNr = expkN[:, 0].astype(np.float64)
    eNi = expkN[:, 1].astype(np.float64)
    annT = (2.0 * (Cp * eNr[None, :] + Sp * eNi[None, :])).astype(bf16)
    amT = (0.5 * (Cp * eMr[None, :] + Sp * eMi[None, :])).astype(bf16)

    def tile_g(t):  # [2048, 2048] -> [2 panels, 128, 16*1024]
        return np.ascontiguousarray(
            t.reshape(_KT, 128, 2, 1024).transpose(2, 1, 0, 3)
            .reshape(2, 128, _KT * 1024))

    def tile_h(t):  # [2048, 2048] -> [4 panels, 128, 16*512]
        return np.ascontiguousarray(
            t.reshape(_KT, 128, 4, 512).transpose(2, 1, 0, 3)
            .reshape(4, 128, _KT * 512))

    tabs = {
        "g2": tile_g(amT[:_H, 0::2]),
        "g4": tile_g(amT[:_H, 1::2]),
        "h2": tile_h(annT[:_H, 0::2]),
        "h4": tile_h(annT[:_H, 1::2]),
        "jrev": np.ascontiguousarray(np.eye(128, dtype=bf16)[::-1]),
    }
    _state["tables"] = (key, tabs)
    return tabs


def kernel(x, expkM, expkN, M, N):
    import ml_dtypes
    x = np.asarray(x, dtype=np.float32)
    expkM = np.asarray(expkM, dtype=np.float32)
    expkN = np.asarray(expkN, dtype=np.float32)
    assert x.shape == (_SZ, _SZ)

    tabs = _tables(expkM, expkN)
    if "run" not in _state:
        _state["run"] = _build_runner(_build_bass(), _NCORES)
    run = _state["run"]

    ins = dict(tabs)
    ins["xc"] = x.astype(ml_dtypes.bfloat16)
    out = run(ins, cache_names=("g2", "g4", "h2", "h4", "jrev"))
    return np.asarray(out).astype(np.float32)
